# revision 3
# baseline (speedup 1.0000x reference)
"""Trainium2 Bass kernel for nn_DebugBertSelfAttention_87093346828836.

The reference module is a debug variant of BERT self-attention: after the
Q/K/V projections it overwrites q, k, v with the constant 0.01.  With
uniform q/k, every attention score is identical, so softmax yields uniform
probabilities (1/SEQ), and the context is the mean of the constant v —
i.e. every output element equals the same f32 constant, independent of all
inputs.  The f32-accumulated value (matching the XLA CPU reference) is
0x3c23d739 = 0.010000044.

The kernel therefore reduces to materializing the (8, 1024, 1024) constant
output.  Sharding: batch across the 8 cores — each core writes one
1024x1024 f32 block (4 MiB).  On device: DVE memsets an SBUF tile with the
constant, then HWDGE DMAs replicate it into the core's output DRAM buffer.
The host concatenates the 8 per-core blocks into the full output.
"""

import numpy as np

NUM_CORES = 8
BATCH, SEQ, HIDDEN = 8, 1024, 1024
OUT_SHAPE = (BATCH, SEQ, HIDDEN)

# Per-core output block: 1024*1024 f32 = 4 MiB, viewed as [128, 8192].
P = 128
F = (SEQ * HIDDEN) // P  # 8192

# SBUF staging tile: [128, CHUNK] f32, replicated F//CHUNK times by DMA.
CHUNK = 1024

# f32 bits of the reference output constant (see module docstring).
CONST_BITS = 0x3C23D739
CONST = float(np.uint32(CONST_BITS).view(np.float32))


def build_nc():
    """Build the per-core Bass program (identical on all cores)."""
    from concourse import bass
    from concourse import mybir

    nc = bass.Bass(target_bir_lowering=False)
    out = nc.dram_tensor("out", [P, F], mybir.dt.float32, kind="ExternalOutput")

    n = F // CHUNK  # total DMA count
    blk = P * CHUNK  # elements per DMA; each DMA writes a contiguous range

    with (
        nc.semaphore("msem") as msem,
        nc.semaphore("dsem") as dsem,
        nc.sbuf_tensor("buf", [P, CHUNK], mybir.dt.float32) as buf,
    ):
        # GPSIMD frees earliest after the framework preamble.
        nc.gpsimd.memset(buf[:, :], CONST).then_inc(msem, 1)

        # Split DMA issue across both HWDGE engines (SP + ACT).  Each DMA
        # writes a fully contiguous 512 KiB DRAM range (partition p of the
        # source lands at byte offset p*CHUNK*4) — sequential HBM addresses
        # instead of 4 KiB writes at 32 KiB stride.
        nc.sync.wait_ge(msem, 1)
        nc.scalar.wait_ge(msem, 1)
        for j in range(n):
            eng = nc.sync if j % 2 == 0 else nc.scalar
            dst = bass.AP(out, j * blk, [[CHUNK, P], [1, CHUNK]])
            eng.dma_start(dst, buf[:, :]).then_inc(dsem, 16)
        nc.sync.wait_ge(dsem, 16 * n)

    return nc


def kernel(**inputs) -> np.ndarray:
    from concourse.bass_utils import run_bass_kernel_spmd

    nc = build_nc()
    in_maps = [{} for _ in range(NUM_CORES)]
    res = run_bass_kernel_spmd(nc, in_maps, list(range(NUM_CORES)))

    out = np.empty(OUT_SHAPE, np.float32)
    for i in range(NUM_CORES):
        out[i] = res.results[i]["out"].reshape(SEQ, HIDDEN)
    return out


# revision 4
# speedup vs baseline: 1.0154x; 1.0154x over previous
"""Trainium2 Bass kernel for nn_DebugBertSelfAttention_87093346828836.

The reference module is a debug variant of BERT self-attention: after the
Q/K/V projections it overwrites q, k, v with the constant 0.01.  With
uniform q/k, every attention score is identical, so softmax yields uniform
probabilities (1/SEQ), and the context is the mean of the constant v —
i.e. every output element equals the same f32 constant, independent of all
inputs.  The f32-accumulated value (matching the XLA CPU reference) is
0x3c23d739 = 0.010000044.

The kernel therefore reduces to materializing the (8, 1024, 1024) constant
output.  Sharding: batch across the 8 cores — each core writes one
1024x1024 f32 block (4 MiB).  On device: DVE memsets an SBUF tile with the
constant, then HWDGE DMAs replicate it into the core's output DRAM buffer.
The host concatenates the 8 per-core blocks into the full output.
"""

import numpy as np

NUM_CORES = 8
BATCH, SEQ, HIDDEN = 8, 1024, 1024
OUT_SHAPE = (BATCH, SEQ, HIDDEN)

# Per-core output block: 1024*1024 f32 = 4 MiB, viewed as [128, 8192].
P = 128
F = (SEQ * HIDDEN) // P  # 8192

# SBUF staging tile: [128, CHUNK] f32, replicated F//CHUNK times by DMA.
CHUNK = 1024

# f32 bits of the reference output constant (see module docstring).
CONST_BITS = 0x3C23D739
CONST = float(np.uint32(CONST_BITS).view(np.float32))


def build_nc():
    """Build the per-core Bass program (identical on all cores)."""
    from concourse import bass
    from concourse import mybir

    nc = bass.Bass(target_bir_lowering=False)
    out = nc.dram_tensor("out", [P, F], mybir.dt.float32, kind="ExternalOutput")

    n = F // CHUNK  # total DMA count
    blk = P * CHUNK  # elements per DMA; each DMA writes a contiguous range

    with (
        nc.semaphore("msem") as msem,
        nc.semaphore("dsem") as dsem,
        nc.sbuf_tensor("buf", [P, CHUNK], mybir.dt.float32) as buf,
    ):
        # GPSIMD frees earliest after the framework preamble.  Memset the
        # staging tile in a ladder of pieces so the first DMAs can start
        # before the whole tile is filled.
        pieces = [128, 128, 256, 512]  # cumulative: 128, 256, 512, 1024
        col = 0
        for w in pieces:
            nc.gpsimd.memset(buf[:, col : col + w], CONST).then_inc(msem, 1)
            col += w

        # Each DMA writes a fully contiguous DRAM byte range (partition p of
        # the source lands at offset p*width*4 within the block) — sequential
        # HBM addresses instead of 4 KiB writes at 32 KiB stride.  Issue is
        # split across both HWDGE engines (SP + ACT).
        # Ladder DMAs: piece i is shipped as soon as memset i lands.
        # Bulk DMAs: 7 full-tile copies fill the rest of the 4 MiB block.
        engines = [nc.sync, nc.scalar]
        transfers = []  # (src_col, width, msem_threshold)
        col = 0
        for i, w in enumerate(pieces):
            transfers.append((col, w, i + 1))
            col += w
        for _ in range(7):
            transfers.append((0, CHUNK, len(pieces)))

        waited = {id(nc.sync): 0, id(nc.scalar): 0}
        off = 0  # output offset in elements
        ndma = 0
        for k, (src_col, w, thresh) in enumerate(transfers):
            eng = engines[k % 2]
            if waited[id(eng)] < thresh:
                eng.wait_ge(msem, thresh)
                waited[id(eng)] = thresh
            dst = bass.AP(out, off, [[w, P], [1, w]])
            eng.dma_start(dst, buf[:, src_col : src_col + w]).then_inc(dsem, 16)
            off += P * w
            ndma += 1
        assert off == P * F
        nc.sync.wait_ge(dsem, 16 * ndma)

    return nc


def kernel(**inputs) -> np.ndarray:
    from concourse.bass_utils import run_bass_kernel_spmd

    nc = build_nc()
    in_maps = [{} for _ in range(NUM_CORES)]
    res = run_bass_kernel_spmd(nc, in_maps, list(range(NUM_CORES)))

    out = np.empty(OUT_SHAPE, np.float32)
    for i in range(NUM_CORES):
        out[i] = res.results[i]["out"].reshape(SEQ, HIDDEN)
    return out


# revision 14
# speedup vs baseline: 1.1235x; 1.1064x over previous
"""Trainium2 Bass kernel for nn_DebugBertSelfAttention_87093346828836.

The reference module is a debug variant of BERT self-attention: after the
Q/K/V projections it overwrites q, k, v with the constant 0.01.  With
uniform q/k, every attention score is identical, so softmax yields uniform
probabilities (1/SEQ), and the context is the mean of the constant v —
i.e. every output element equals the same f32 constant, independent of all
inputs.  The f32-accumulated value (matching the XLA CPU reference) is
0x3c23d739 = 0.010000044.

The kernel therefore reduces to materializing the (8, 1024, 1024) constant
output.  Sharding: batch across the 8 cores — each core writes one
1024x1024 f32 block (4 MiB).  On device: DVE memsets an SBUF tile with the
constant, then HWDGE DMAs replicate it into the core's output DRAM buffer.
The host concatenates the 8 per-core blocks into the full output.
"""

import numpy as np

NUM_CORES = 8
BATCH, SEQ, HIDDEN = 8, 1024, 1024
OUT_SHAPE = (BATCH, SEQ, HIDDEN)

# Per-core output block: 1024*1024 f32 = 4 MiB, viewed as [128, 8192].
P = 128
F = (SEQ * HIDDEN) // P  # 8192

# SBUF staging tile: [128, CHUNK] f32, replicated F//CHUNK times by DMA.
CHUNK = 1024

# f32 bits of the reference output constant (see module docstring).
CONST_BITS = 0x3C23D739
CONST = float(np.uint32(CONST_BITS).view(np.float32))


VARIANT = "half"  # default variant used by kernel()


def build_nc(variant=None):
    """Build the per-core Bass program (identical on all cores)."""
    from concourse import bass
    from concourse import mybir

    variant = variant or VARIANT
    nc = bass.Bass(target_bir_lowering=False)
    out = nc.dram_tensor("out", [P, F], mybir.dt.float32, kind="ExternalOutput")

    # Staging tile width and memset pieces per variant.
    if variant == "empty":
        chunk, pieces = CHUNK, []  # wrapper-floor probe: no body at all
    elif variant in ("simple", "split"):
        chunk, pieces = CHUNK, [CHUNK]
    elif variant == "ladder":
        chunk, pieces = CHUNK, [128, 128, 256, 512]
    elif variant == "half":
        chunk, pieces = CHUNK, [512, 512]
    elif variant == "big":
        chunk, pieces = 2048, [1024, 1024]
    elif variant == "big4":
        chunk, pieces = 4096, [1024, 1024, 2048]
    elif variant == "tailsplit":
        chunk, pieces = 1024, [512, 512]
    else:
        raise ValueError(variant)

    with (
        nc.semaphore("msem") as msem,
        nc.semaphore("dsem") as dsem,
        nc.sbuf_tensor("buf", [P, chunk], mybir.dt.float32) as buf,
    ):
        if variant == "empty":
            return nc
        # GPSIMD frees earliest after the framework preamble.  Memset the
        # staging tile, optionally in pieces so the first DMAs can start
        # before the whole tile is filled.
        assert sum(pieces) == chunk
        col = 0
        for w in pieces:
            nc.gpsimd.memset(buf[:, col : col + w], CONST).then_inc(msem, 1)
            col += w

        # Each DMA writes a fully contiguous DRAM byte range (partition p of
        # the source lands at offset p*width*4 within the block) — sequential
        # HBM addresses instead of 4 KiB writes at 32 KiB stride.  Issue is
        # split across both HWDGE engines (SP + ACT).
        # Ladder DMAs ship piece i as soon as memset i lands; bulk DMAs copy
        # the full tile to fill the rest of the 4 MiB block.
        engines = [nc.sync, nc.scalar]
        transfers = []  # (src_col, width, msem_threshold)
        col = 0
        for i, w in enumerate(pieces):
            transfers.append((col, w, i + 1))
            col += w
        n_bulk = (F - chunk) // chunk
        for _ in range(n_bulk):
            transfers.append((0, chunk, len(pieces)))
        if variant == "tailsplit":
            # Replace the final bulk DMA with quarters so the last write
            # receipts pipeline instead of one 512 KiB receipt at the end.
            transfers.pop()
            transfers += [(c, 256, len(pieces)) for c in (0, 256, 512, 768)]

        waited = {id(nc.sync): 0, id(nc.scalar): 0}
        off = 0  # output offset in elements
        ndma = 0
        for k, (src_col, w, thresh) in enumerate(transfers):
            if variant == "split":
                # Each engine streams a contiguous half of the output.
                eng = engines[0] if k < len(transfers) // 2 else engines[1]
            else:
                eng = engines[k % 2]
            if waited[id(eng)] < thresh:
                eng.wait_ge(msem, thresh)
                waited[id(eng)] = thresh
            dst = bass.AP(out, off, [[w, P], [1, w]])
            eng.dma_start(dst, buf[:, src_col : src_col + w]).then_inc(dsem, 16)
            off += P * w
            ndma += 1
        assert off == P * F
        nc.sync.wait_ge(dsem, 16 * ndma)

    return nc


def kernel(**inputs) -> np.ndarray:
    from concourse.bass_utils import run_bass_kernel_spmd

    last_err = None
    for _attempt in range(3):
        try:
            nc = build_nc()
            in_maps = [{} for _ in range(NUM_CORES)]
            res = run_bass_kernel_spmd(nc, in_maps, list(range(NUM_CORES)))
            out = np.empty(OUT_SHAPE, np.float32)
            for i in range(NUM_CORES):
                shard = np.asarray(res.results[i]["out"])
                if not (shard == np.float32(CONST)).all():
                    raise RuntimeError(f"core {i} returned corrupt shard")
                out[i] = shard.reshape(SEQ, HIDDEN)
            return out
        except Exception as e:  # transient NRT wedges: retry on a fresh run
            last_err = e
    raise last_err
